# revision 5
# baseline (speedup 1.0000x reference)
"""CosLoss (ArcFace-style margin loss) Trainium2 kernel, 8-way class-sharded.

Math (reference):
    xn   = x / ||x||_row                       [B, D]
    wf   = xn @ W.T                            [B, C]
    corr = wf[i, labels[i]]                    [B]
    num  = S*(corr - M)
    excl = sum_j exp(S*wf[i,j]) - exp(S*corr)
    L    = num - log(exp(num) + excl);  out = -mean(L)

Sharding: classes split across 8 cores (4000 each, zero-padded to 4096).
Each core computes, for ALL B rows x its class shard:
    rowsum_c[i] = sum_{j in shard} exp(rs_i * z[i,j]),  rs_i = S/||x_i||,
    z = x @ W_shard.T  (bf16 matmul, fp32 PSUM accumulate)
plus, for its 1/8 slice of the batch, the exact fp32 dot
    dotg[i] = x_i . W[labels[i]]  (W rows gathered on host).
Host combines: rowsum = sum_c rowsum_c - pad_correction, corr = rs*dotg,
then the scalar loss. Heavy work (134 GFLOP matmul + 262M exps) is on-device;
host does only O(B) glue.
"""

import os
from contextlib import ExitStack

import ml_dtypes
import numpy as np

import concourse.bass as bass
import concourse.mybir as mybir
import concourse.tile as tile
from concourse import bacc
from concourse.bass_utils import run_bass_kernel_spmd

S = 30.0
MARGIN = 0.4
N_CORES = 8
B, D, C = 8192, 256, 32000
CSH = C // N_CORES          # 4000 real classes per core
CPAD = 4096                 # padded shard width (bank-aligned halves of 2048)
BSH = B // N_CORES          # 1024 batch rows per core for the correct-logit dot
P = 128

FP32 = mybir.dt.float32
BF16 = mybir.dt.bfloat16


def _emit(tc, ins, outs, b, d, cpad, bsh):
    """Per-core program. All per-core differences arrive via input data."""
    nc = tc.nc
    xT, wT, xf, xg, wg = ins["xT"], ins["wT"], ins["xf"], ins["xg"], ins["wg"]
    rowsum, ssq_out, dotg_out = outs["rowsum"], outs["ssq"], outs["dotg"]

    kk_n = d // P               # contraction tiles (2)
    nbt = b // P                # batch tiles (64)
    ng = bsh // P               # gather tiles (8)
    half = cpad // 2            # classes per PSUM tile (2048 = 4 banks fp32)
    nj = (half + 511) // 512    # matmuls per PSUM tile

    with ExitStack() as ctx:
        singles = ctx.enter_context(tc.tile_pool(name="singles", bufs=1))
        work = ctx.enter_context(tc.tile_pool(name="work", bufs=3))
        scr = ctx.enter_context(tc.tile_pool(name="scr", bufs=2))
        stats = ctx.enter_context(tc.tile_pool(name="stats", bufs=4))
        psum = ctx.enter_context(tc.tile_pool(name="psum", bufs=2, space="PSUM"))

        # Resident operands: xT (stationary), wT (moving), both bf16.
        xT_sb = singles.tile([P, kk_n, b], BF16)
        nc.sync.dma_start(out=xT_sb, in_=xT.rearrange("(kk p) b -> p kk b", p=P))
        wT_sb = singles.tile([P, kk_n, cpad], BF16)
        nc.sync.dma_start(out=wT_sb, in_=wT.rearrange("(kk p) c -> p kk c", p=P))

        # Phase 1: row sum-of-squares for every batch row; rs = S/||x||.
        ssq_sb = singles.tile([P, nbt], FP32)
        for bt in range(nbt):
            x_t = work.tile([P, d], FP32, tag="x_t")
            nc.sync.dma_start(out=x_t, in_=xf[bt * P : (bt + 1) * P, :])
            sq = scr.tile([P, d], FP32, tag="sq")
            nc.vector.tensor_mul(out=sq, in0=x_t, in1=x_t)
            nc.vector.reduce_sum(
                out=ssq_sb[:, bt : bt + 1], in_=sq, axis=mybir.AxisListType.X
            )
        nc.sync.dma_start(out=ssq_out.rearrange("(t p) -> p t", p=P), in_=ssq_sb)
        rinv = singles.tile([P, nbt], FP32)
        nc.vector.reciprocal(out=rinv, in_=ssq_sb)
        rs_all = singles.tile([P, nbt], FP32)
        nc.scalar.activation(
            out=rs_all, in_=rinv, func=mybir.ActivationFunctionType.Sqrt,
            scale=S * S,
        )

        # Phase 1b: exact fp32 dot x_i . W[l_i] for this core's batch slice.
        dotg_sb = singles.tile([P, ng], FP32)
        for j in range(ng):
            xg_t = work.tile([P, d], FP32, tag="xg_t")
            nc.sync.dma_start(out=xg_t, in_=xg[j * P : (j + 1) * P, :])
            wg_t = work.tile([P, d], FP32, tag="wg_t")
            nc.sync.dma_start(out=wg_t, in_=wg[j * P : (j + 1) * P, :])
            dg = scr.tile([P, d], FP32, tag="sq")
            nc.vector.tensor_mul(out=dg, in0=xg_t, in1=wg_t)
            nc.vector.reduce_sum(
                out=dotg_sb[:, j : j + 1], in_=dg, axis=mybir.AxisListType.X
            )
        nc.sync.dma_start(out=dotg_out.rearrange("(t p) -> p t", p=P), in_=dotg_sb)

        # Phase 2: z = x @ Wshard.T in 2048-class PSUM tiles; fused
        # exp(rs*z) + free-axis row-sum on ACT via accum_out.
        rsum_sb = singles.tile([P, nbt], FP32)
        for bt in range(nbt):
            parts = stats.tile([P, 2], FP32, tag="parts")
            for h in range(2):
                pt = psum.tile([P, half], FP32, tag="pt")
                for j in range(nj):
                    c0 = j * 512
                    cw = min(512, half - c0)
                    for kk in range(kk_n):
                        nc.tensor.matmul(
                            pt[:, c0 : c0 + cw],
                            lhsT=xT_sb[:, kk, bt * P : (bt + 1) * P],
                            rhs=wT_sb[:, kk, h * half + c0 : h * half + c0 + cw],
                            start=(kk == 0),
                            stop=(kk == kk_n - 1),
                        )
                et = scr.tile([P, half], BF16, tag="et")
                nc.scalar.activation(
                    out=et, in_=pt, func=mybir.ActivationFunctionType.Exp,
                    scale=rs_all[:, bt : bt + 1],
                    accum_out=parts[:, h : h + 1],
                )
            nc.vector.tensor_add(
                out=rsum_sb[:, bt : bt + 1],
                in0=parts[:, 0:1], in1=parts[:, 1:2],
            )
        nc.sync.dma_start(out=rowsum.rearrange("(t p) -> p t", p=P), in_=rsum_sb)


def _build(b=B, d=D, cpad=CPAD, bsh=BSH):
    nc = bacc.Bacc("TRN2", target_bir_lowering=False, debug=False)
    ins = {
        "xT": nc.dram_tensor("xT", [d, b], BF16, kind="ExternalInput").ap(),
        "wT": nc.dram_tensor("wT", [d, cpad], BF16, kind="ExternalInput").ap(),
        "xf": nc.dram_tensor("xf", [b, d], FP32, kind="ExternalInput").ap(),
        "xg": nc.dram_tensor("xg", [bsh, d], FP32, kind="ExternalInput").ap(),
        "wg": nc.dram_tensor("wg", [bsh, d], FP32, kind="ExternalInput").ap(),
    }
    outs = {
        "rowsum": nc.dram_tensor("rowsum", [b], FP32, kind="ExternalOutput").ap(),
        "ssq": nc.dram_tensor("ssq", [b], FP32, kind="ExternalOutput").ap(),
        "dotg": nc.dram_tensor("dotg", [bsh], FP32, kind="ExternalOutput").ap(),
    }
    with tile.TileContext(nc) as tc:
        _emit(tc, ins, outs, b, d, cpad, bsh)
    nc.compile()
    return nc


_NC_CACHE = {}


def _get_nc():
    if "nc" not in _NC_CACHE:
        _NC_CACHE["nc"] = _build()
    return _NC_CACHE["nc"]


def _install_trace_hook():
    """Make `antenv.axon_hooks` importable so run_bass_kernel_spmd(trace=True)
    can capture NTFF profiles under axon. Returns False if unavailable."""
    try:
        from antenv.axon_hooks import get_axon_ntff_profile_hook  # noqa: F401

        return True
    except ImportError:
        pass
    try:
        import sys
        import types

        from trn_agent_boot.trn_boot import _ntff_profile_via_ctypes

        hook = _ntff_profile_via_ctypes("/opt/axon/libaxon_pjrt.so")
        if hook is None:
            return False
        mod = types.ModuleType("antenv.axon_hooks")
        mod._hook = hook
        mod.get_axon_ntff_profile_hook = lambda: mod._hook
        mod.set_axon_ntff_profile_hook = lambda h: setattr(mod, "_hook", h)
        sys.modules["antenv.axon_hooks"] = mod
        import antenv

        antenv.axon_hooks = mod
        return True
    except Exception:
        return False


def kernel(x, labels, W, trace=False):
    x = np.ascontiguousarray(np.asarray(x, dtype=np.float32))
    W = np.ascontiguousarray(np.asarray(W, dtype=np.float32))
    labels_i = np.asarray(labels).astype(np.int64)

    xT_bf = np.ascontiguousarray(x.T).astype(ml_dtypes.bfloat16)

    in_maps = []
    for k in range(N_CORES):
        wTk = np.zeros((D, CPAD), dtype=ml_dtypes.bfloat16)
        wTk[:, :CSH] = W[k * CSH : (k + 1) * CSH].T.astype(ml_dtypes.bfloat16)
        lab_k = labels_i[k * BSH : (k + 1) * BSH]
        in_maps.append(
            {
                "xT": xT_bf,
                "wT": wTk,
                "xf": x,
                "xg": np.ascontiguousarray(x[k * BSH : (k + 1) * BSH]),
                "wg": np.ascontiguousarray(W[lab_k]),
            }
        )

    nc = _get_nc()
    if trace and not _install_trace_hook():
        trace = False
    res = run_bass_kernel_spmd(nc, in_maps, core_ids=list(range(N_CORES)), trace=trace)
    if trace and res.exec_time_ns is not None:
        print(f"HW exec time: {res.exec_time_ns} ns")

    rowsum = np.zeros(B, dtype=np.float64)
    for r in res.results:
        rowsum += r["rowsum"].astype(np.float64)
    rowsum -= N_CORES * (CPAD - CSH)  # zero-padded classes contribute exp(0)=1

    ssq = res.results[0]["ssq"].astype(np.float64)
    dotg = np.concatenate([r["dotg"] for r in res.results]).astype(np.float64)

    rs = S / np.sqrt(ssq)                     # [B]
    scorr = rs * dotg                         # S * wf[i, labels[i]]
    num = scorr - S * MARGIN
    excl = rowsum - np.exp(scorr)
    L = num - np.log(np.exp(num) + excl)
    return np.float32(-np.mean(L))
